# revision 5
# baseline (speedup 1.0000x reference)
"""Trainium2 Bass kernel for nn_CrossAttention_33423435498049.

The reference broadcasts age_features across the sequence dimension
*before* the K/V projections, so every K row (and every V row) within a
batch is identical. Scores are therefore constant along the softmax
axis, softmax is exactly uniform, and the attention output collapses to
the single V row:

    out[b, n, :] = pixel_features[b, n, :] + (age_features[b, :] @ Wv + bv)

This holds for all input values (not just a particular seed); the Wq/bq
and Wk/bk parameters cannot affect the output. The kernel computes the
collapsed form on-device: batch is sharded 1-per-core across 8 cores,
each core runs a tiny [128x1]^T @ [128x768] matmul for its V row and a
DMA-bound broadcast-add over its [2048, 768] pixel slab.
"""

import numpy as np

B, N, D, A = 8, 2048, 768, 128
P = 128                 # SBUF partitions
R = 4                   # rows of D packed per partition per tile
TILE_F = R * D          # free-dim elements per tile
N_TILES = N // (P * R)  # row-tiles per core

_CACHE = {}


def _build_bass():
    import concourse.mybir as mybir
    from concourse.bacc import Bacc
    from concourse.tile import TileContext

    f32 = mybir.dt.float32
    nc = Bacc()

    pixel = nc.dram_tensor("pixel", [N, D], f32, kind="ExternalInput")
    age = nc.dram_tensor("age", [A, 1], f32, kind="ExternalInput")
    wv = nc.dram_tensor("wv", [A, D], f32, kind="ExternalInput")
    bv = nc.dram_tensor("bv", [1, D], f32, kind="ExternalInput")
    out = nc.dram_tensor("out", [N, D], f32, kind="ExternalOutput")

    pixel_t = pixel.rearrange("(t p r) d -> t p (r d)", p=P, r=R)
    out_t = out.rearrange("(t p r) d -> t p (r d)", p=P, r=R)

    with TileContext(nc) as tc:
        with (
            tc.tile_pool(name="const", bufs=1) as const,
            tc.tile_pool(name="io", bufs=N_TILES) as io,
            tc.tile_pool(name="psum", bufs=1, space="PSUM") as psum,
        ):
            # v_row = age @ Wv + bv, materialized pre-broadcast across all
            # 128 partitions by using an age tile whose free dim replicates
            # the vector: out[m, n] = sum_k age[k] * Wv[k, n] for every m.
            age_sb = const.tile([A, 1], f32)
            nc.sync.dma_start(out=age_sb[:], in_=age[:])
            wv_sb = const.tile([A, D], f32)
            nc.sync.dma_start(out=wv_sb[:], in_=wv[:])
            bv_bc = const.tile([P, D], f32)
            nc.gpsimd.dma_start(out=bv_bc[:], in_=bv[:].to_broadcast((P, D)))

            age_bc = const.tile([A, P], f32)
            nc.vector.tensor_copy(out=age_bc[:], in_=age_sb[:].to_broadcast((A, P)))

            v_psum = psum.tile([P, D], f32)
            nc.tensor.matmul(v_psum[:, 0:512], age_bc[:], wv_sb[:, 0:512])
            nc.tensor.matmul(v_psum[:, 512:D], age_bc[:], wv_sb[:, 512:D])

            # vbc holds R copies of (v_row + bv) so the hot-loop add is a
            # single stride-1 tensor_tensor per tile.
            vbc = const.tile([P, TILE_F], f32)
            nc.vector.tensor_add(out=vbc[:, 0:D], in0=v_psum[:], in1=bv_bc[:])
            rep = 1
            while rep < R:
                w = min(rep, R - rep) * D
                nc.vector.tensor_copy(
                    out=vbc[:, rep * D : rep * D + w], in_=vbc[:, 0:w]
                )
                rep += min(rep, R - rep)

            for i in range(N_TILES):
                tile = io.tile([P, TILE_F], f32)
                nc.sync.dma_start(out=tile[:], in_=pixel_t[i])
                nc.vector.tensor_add(out=tile[:], in0=tile[:], in1=vbc[:])
                nc.sync.dma_start(out=out_t[i], in_=tile[:])

    nc.finalize()
    return nc


def _get_bass():
    if "nc" not in _CACHE:
        _CACHE["nc"] = _build_bass()
    return _CACHE["nc"]


def _run(inputs, **spmd_kwargs):
    from concourse.bass_utils import run_bass_kernel_spmd

    pixel = np.ascontiguousarray(np.asarray(inputs["pixel_features"], np.float32))
    age = np.ascontiguousarray(np.asarray(inputs["age_features"], np.float32))
    Wv = np.ascontiguousarray(np.asarray(inputs["Wv"], np.float32))
    bv = np.ascontiguousarray(np.asarray(inputs["bv"], np.float32)).reshape(1, D)

    nc = _get_bass()
    in_maps = [
        {
            "pixel": pixel[b],
            "age": age[b].reshape(A, 1),
            "wv": Wv,
            "bv": bv,
        }
        for b in range(B)
    ]
    res = run_bass_kernel_spmd(nc, in_maps, list(range(B)), **spmd_kwargs)
    return np.stack([res.results[b]["out"] for b in range(B)], axis=0), res


def kernel(**inputs) -> np.ndarray:
    return _run(inputs)[0]


# revision 6
# speedup vs baseline: 1.0507x; 1.0507x over previous
"""Trainium2 Bass kernel for nn_CrossAttention_33423435498049.

The reference broadcasts age_features across the sequence dimension
*before* the K/V projections, so every K row (and every V row) within a
batch is identical. Scores are therefore constant along the softmax
axis, softmax is exactly uniform, and the attention output collapses to
the single V row:

    out[b, n, :] = pixel_features[b, n, :] + (age_features[b, :] @ Wv + bv)

This holds for all input values (not just a particular seed); the Wq/bq
and Wk/bk parameters cannot affect the output. The kernel computes the
collapsed form on-device: batch is sharded 1-per-core across 8 cores.
Each core runs a tiny [128x1]^T @ [128x768] matmul for its V row (age is
packed as column 768 of the wvx const DMA so the const path is a single
early transfer) and a DMA-bound broadcast-add over its [2048, 768] pixel
slab. Raw bacc engine blocks with manual semaphores; pixel loads stream
on the sync HWDGE ring while stores go out on the scalar ring. Measured
~44.5 us/core on trn2 against a ~36 us HBM floor for the 12 MB of
mandatory traffic plus ~7 us fixed engine-start/barrier overhead.
"""

import numpy as np

B, N, D, A = 8, 2048, 768, 128
P = 128                 # SBUF partitions
R = 4                   # rows of D packed per partition per tile
TILE_F = R * D          # free-dim elements per tile
T = N // (P * R)        # row-tiles per core

_CACHE = {}


def _build_bass():
    from contextlib import ExitStack

    import concourse.mybir as mybir
    from concourse.bacc import Bacc

    f32 = mybir.dt.float32
    nc = Bacc()

    pixel = nc.dram_tensor("pixel", [N, D], f32, kind="ExternalInput")
    wvx = nc.dram_tensor("wvx", [A, D + 1], f32, kind="ExternalInput")
    bv = nc.dram_tensor("bv", [1, D], f32, kind="ExternalInput")
    out = nc.dram_tensor("out", [N, D], f32, kind="ExternalOutput")

    pixel_t = pixel.rearrange("(t p r) d -> t p (r d)", p=P, r=R)
    out_t = out.rearrange("(t p r) d -> t p (r d)", p=P, r=R)

    with ExitStack() as ctx:
        wvx_sb = ctx.enter_context(nc.sbuf_tensor("wvx_sb", [A, D + 1], f32))
        bv_bc = ctx.enter_context(nc.sbuf_tensor("bv_bc", [P, D], f32))
        age_bc = ctx.enter_context(nc.sbuf_tensor("age_bc", [A, P], f32))
        vbc = ctx.enter_context(nc.sbuf_tensor("vbc", [P, D], f32))
        tiles = [
            ctx.enter_context(nc.sbuf_tensor(f"t{i}", [P, TILE_F], f32))
            for i in range(T)
        ]
        v_psum = ctx.enter_context(nc.psum_tensor("v_psum", [P, D], f32))

        cs = ctx.enter_context(nc.semaphore("cs"))
        bs = ctx.enter_context(nc.semaphore("bs"))
        vc = ctx.enter_context(nc.semaphore("vc"))
        pe = ctx.enter_context(nc.semaphore("pe"))
        vb = ctx.enter_context(nc.semaphore("vb"))
        as_ = ctx.enter_context(nc.semaphore("as"))
        ss = ctx.enter_context(nc.semaphore("ss"))
        ls = [ctx.enter_context(nc.semaphore(f"ls{i}")) for i in range(T)]

        block = ctx.enter_context(nc.Block())

        @block.sync
        def _(sync):
            sync.dma_start(out=wvx_sb[:], in_=wvx[:]).then_inc(cs, 16)
            for i in range(T):
                sync.dma_start(out=tiles[i][:], in_=pixel_t[i]).then_inc(ls[i], 16)

        @block.scalar
        def _(scalar):
            for i in range(T):
                scalar.wait_ge(as_, i + 1)
                scalar.dma_start(out=out_t[i], in_=tiles[i][:]).then_inc(ss, 16)
            scalar.wait_ge(ss, 16 * T)

        @block.gpsimd
        def _(gpsimd):
            gpsimd.dma_start(out=bv_bc[:], in_=bv[:].to_broadcast((P, D))).then_inc(
                bs, 16
            )

        @block.vector
        def _(vector):
            vector.wait_ge(cs, 16)
            vector.tensor_copy(
                out=age_bc[:], in_=wvx_sb[:, D : D + 1].to_broadcast((A, P))
            ).then_inc(vc, 1)
            vector.wait_ge(bs, 16)
            vector.wait_ge(pe, 1)
            vector.tensor_add(out=vbc[:], in0=v_psum[:], in1=bv_bc[:]).then_inc(
                vb, 1
            )
            for i in range(T):
                vector.wait_ge(vb, 1)
                vector.wait_ge(ls[i], 16)
                t3 = tiles[i][:].rearrange("p (r d) -> p r d", d=D)
                vector.tensor_add(
                    out=t3, in0=t3, in1=vbc[:, None, :].to_broadcast((P, R, D))
                ).then_inc(as_, 1)

        @block.tensor
        def _(tensor):
            tensor.wait_ge(vc, 1)
            tensor.matmul(v_psum[:, 0:512], age_bc[:], wvx_sb[:, 0:512])
            tensor.matmul(v_psum[:, 512:D], age_bc[:], wvx_sb[:, 512:D]).then_inc(
                pe, 1
            )

    nc.finalize()
    return nc


def _get_bass():
    if "nc" not in _CACHE:
        _CACHE["nc"] = _build_bass()
    return _CACHE["nc"]


def _run(inputs, **spmd_kwargs):
    from concourse.bass_utils import run_bass_kernel_spmd

    pixel = np.ascontiguousarray(np.asarray(inputs["pixel_features"], np.float32))
    age = np.ascontiguousarray(np.asarray(inputs["age_features"], np.float32))
    Wv = np.ascontiguousarray(np.asarray(inputs["Wv"], np.float32))
    bv = np.ascontiguousarray(np.asarray(inputs["bv"], np.float32)).reshape(1, D)

    nc = _get_bass()
    in_maps = [
        {
            "pixel": pixel[b],
            "wvx": np.ascontiguousarray(
                np.concatenate([Wv, age[b][:, None]], axis=1)
            ),
            "bv": bv,
        }
        for b in range(B)
    ]
    res = run_bass_kernel_spmd(nc, in_maps, list(range(B)), **spmd_kwargs)
    return np.stack([res.results[b]["out"] for b in range(B)], axis=0), res


def kernel(**inputs) -> np.ndarray:
    return _run(inputs)[0]
